# revision 10
# baseline (speedup 1.0000x reference)
"""BiMamba forward kernel for 8 TRN2 NeuronCores.

Sharding: core c = (batch b, direction dir, d_inner half h); the host
pre-flips reverse-direction inputs in time so the device program is
identical (purely causal) on all cores. Each core produces a partial
output projection [d_model, L]; the host sums four partials per batch
element (unflipping the reverse ones). A host-side channel permutation
puts this core's d_inner half in x-path tiles 0..5 so the single SPMD
program needs no per-core branches.

Device layout: channels on partitions, time on the free dim. Each core
computes the in-proj/conv for the FULL d_inner locally (PE has slack)
so x_dbl needs no collective. The scan is hardware tensor_tensor_scan
(h = dA*h + dBu along time), one instruction per (128-channel tile,
state s). All elementwise work stays on VectorE: running tensor_tensor
on GpSimd concurrently halves DVE throughput (shared SBUF ports), so
the Pool engine is left idle on purpose. Decay planes dA_s come from
ScalarE exp (per-partition scale = A[:,s]); delta is a single Softplus
activation straight from PSUM. sum_s + Dp-skip accumulate in PSUM via
identity/diagonal matmuls on TensorE; the causal conv also runs on
TensorE as 4 diagonal matmuls over shifted views. B/C rows bounce
through DRAM and return as partition-broadcast DMA reads forming
[128, L] replicated tiles.
"""
import numpy as np
import ml_dtypes

import concourse.bass as bass
import concourse.tile as tile
from concourse import bacc, mybir
from concourse.bass_utils import run_bass_kernel_spmd

D_MODEL = 768
D_INNER = 1536
D_STATE = 16
D_CONV = 4
DT_RANK = 48
BATCH = 2
SEQLEN = 2048

HALF = D_INNER // 2
NDT = HALF // 128            # 6 half d-tiles
NDT_FULL = D_INNER // 128    # 12 full d-tiles
NK = D_MODEL // 128          # 6 k-tiles over d_model
L = SEQLEN
NCH = 4
CW = L // NCH                # 512
NXD = DT_RANK + 2 * D_STATE  # 80
NXP = 96                     # x_dbl psum rows padded: B/C at partition 64
NM = D_MODEL // 128          # 6 out-proj row tiles

F32 = mybir.dt.float32
BF16 = mybir.dt.bfloat16
BF_NP = ml_dtypes.bfloat16

N_S_F32 = 4                  # fp32 decay planes for slow-decaying states

AF = mybir.ActivationFunctionType
OP = mybir.AluOpType


def build_program(debug_stage=0):
    nc = bacc.Bacc("TRN2", target_bir_lowering=False, debug=False,
                   num_devices=8)
    dram = {}

    def din(name, shape, dt):
        dram[name] = nc.dram_tensor(name, list(shape), dt,
                                    kind="ExternalInput").ap()

    def dout(name, shape, dt):
        dram[name] = nc.dram_tensor(name, list(shape), dt,
                                    kind="ExternalOutput").ap()

    din("uT", (D_MODEL, L), BF16)
    din("w_in_xT", (D_MODEL, D_INNER), BF16)
    din("w_in_zT", (D_MODEL, HALF), BF16)
    din("conv_diag", (NDT_FULL * D_CONV * 128, 128), BF16)
    din("conv_b", (D_INNER, 1), F32)
    din("w_xT", (D_INNER, NXP), BF16)
    din("w_dtT", (DT_RANK, HALF), BF16)
    din("b_dt", (HALF, 1), F32)
    din("A_half", (HALF, D_STATE), F32)
    din("dp_diag", (NDT * 128, 128), BF16)
    din("idn", (128, 128), BF16)
    din("w_outT", (HALF, D_MODEL), BF16)

    if debug_stage == 1:
        dout("xc_dbg", (D_INNER, L), F32)
        dout("delta_dbg", (HALF, L), F32)
        dout("xdbl_dbg", (NXP, L), F32)
    dout("out_part", (D_MODEL, L), F32)

    with tile.TileContext(nc) as tc:
        _body_once(nc, tc, dram, debug_stage)
    nc.compile()
    return nc


def _body_once(nc, tc, dram, dbg):
    with tc.tile_pool(name="wpool", bufs=1) as wp, \
         tc.tile_pool(name="dramp", bufs=1, space="DRAM") as dp_pool:

        # ---- DRAM scratch for B/C partition-broadcast bounce ----
        bc_scr = dp_pool.tile([2 * D_STATE, L], BF16, name="bc_scr")

        # ---- persistent small weights ----
        idn = wp.tile([128, 128], BF16, name="idn")
        nc.sync.dma_start(idn[:], dram["idn"][:])
        dp_diag = [wp.tile([128, 128], BF16, name=f"dpd{r}")
                   for r in range(NDT)]
        A_col = [wp.tile([128, D_STATE], F32, name=f"acol{r}")
                 for r in range(NDT)]
        b_dt = [wp.tile([128, 1], F32, name=f"bdt{r}") for r in range(NDT)]
        conv_b = [wp.tile([128, 1], F32, name=f"cvb{r}")
                  for r in range(NDT_FULL)]
        for r in range(NDT):
            nc.sync.dma_start(dp_diag[r][:],
                              dram["dp_diag"][r * 128:(r + 1) * 128, :])
            nc.sync.dma_start(A_col[r][:],
                              dram["A_half"][r * 128:(r + 1) * 128, :])
            nc.sync.dma_start(b_dt[r][:],
                              dram["b_dt"][r * 128:(r + 1) * 128, :])
        for r in range(NDT_FULL):
            nc.sync.dma_start(conv_b[r][:],
                              dram["conv_b"][r * 128:(r + 1) * 128, :])
        w_dtT = wp.tile([DT_RANK, HALF], BF16, name="w_dtT")
        nc.sync.dma_start(w_dtT[:], dram["w_dtT"][:])
        w_outT = [wp.tile([128, D_MODEL], BF16, name=f"wout{r}")
                  for r in range(NDT)]
        for r in range(NDT):
            nc.sync.dma_start(w_outT[r][:],
                              dram["w_outT"][r * 128:(r + 1) * 128, :])
        w_xT = [wp.tile([128, NXP], BF16, name=f"wx{k}")
                for k in range(NDT_FULL)]
        for k in range(NDT_FULL):
            nc.sync.dma_start(w_xT[k][:],
                              dram["w_xT"][k * 128:(k + 1) * 128, :])

        with tc.tile_pool(name="hold", bufs=1) as hold:
            xdbl_bf = hold.tile([NXP, L], BF16, name="xdbl_bf")
            yg_bf = [hold.tile([128, L], BF16, name=f"yg{r}")
                     for r in range(NDT)]
            xc_own = [hold.tile([128, L], BF16, name=f"xco{r}")
                      for r in range(NDT)]
            gz = [hold.tile([128, L], BF16, name=f"gz{r}")
                  for r in range(NDT)]

            with tc.tile_pool(name="psall", bufs=1,
                              space="PSUM") as psall:
                env = {"hold": hold, "xdbl_bf": xdbl_bf, "yg_bf": yg_bf,
                       "xc_own": xc_own, "gz": gz, "bc_scr": bc_scr,
                       "conv_b": conv_b, "w_xT": w_xT, "w_dtT": w_dtT,
                       "A_col": A_col, "b_dt": b_dt, "dp_diag": dp_diag,
                       "idn": idn, "psall": psall}
                _stages_123(nc, tc, dram, dbg, wp, env)
                _scan_stage(nc, tc, dram, dbg, wp, env)

                # ---------- stage 6: out-proj ----------
                with tc.tile_pool(name="op6", bufs=1) as p6:
                    for m in range(NM):
                        for n in range(NCH):
                            ps = psall.tile([128, CW], F32, name="ps6t",
                                            tag=f"yp{(m * NCH + n) % 8}")
                            for r in range(NDT):
                                nc.tensor.matmul(
                                    ps[:],
                                    w_outT[r][:, m * 128:(m + 1) * 128],
                                    yg_bf[r][:, n * CW:(n + 1) * CW],
                                    start=(r == 0), stop=(r == NDT - 1))
                            ot = p6.tile([128, CW], F32, name="ot",
                                         tag="ot", bufs=4)
                            nc.scalar.copy(ot[:], ps[:])
                            nc.sync.dma_start(
                                dram["out_part"][m * 128:(m + 1) * 128,
                                                 n * CW:(n + 1) * CW],
                                ot[:])


def _stages_123(nc, tc, dram, dbg, wp, env):
    xdbl_bf = env["xdbl_bf"]
    xc_own = env["xc_own"]
    gz = env["gz"]
    conv_b = env["conv_b"]
    w_xT = env["w_xT"]
    bc_scr = env["bc_scr"]
    psall = env["psall"]
    LPAD = L + 3
    psn = [0]

    def psum_tile(rows=128):
        t = psall.tile([rows, CW], F32, name="psp",
                       tag=f"yp{psn[0] % 8}")
        psn[0] += 1
        return t

    with tc.tile_pool(name="pre3", bufs=1) as p3:
        xc_oth = [p3.tile([128, L], BF16, name=f"xoth{r}", tag=f"xoth{r}")
                  for r in range(NDT_FULL - NDT)]
        uT = [p3.tile([128, L], BF16, name=f"uT{k}", tag=f"uT{k}")
              for k in range(NK)]
        for k in range(NK):
            nc.sync.dma_start(uT[k][:],
                              dram["uT"][k * 128:(k + 1) * 128, :])
        w_in_zT = [p3.tile([128, HALF], BF16, name=f"wiz{k}",
                           tag=f"wiz{k}") for k in range(NK)]
        for k in range(NK):
            nc.sync.dma_start(w_in_zT[k][:],
                              dram["w_in_zT"][k * 128:(k + 1) * 128, :])
        with tc.tile_pool(name="pre12", bufs=1) as p12:
            w_in_xT = [p12.tile([128, D_INNER], BF16, name=f"wix{k}",
                                tag=f"wix{k}") for k in range(NK)]
            for k in range(NK):
                nc.sync.dma_start(w_in_xT[k][:],
                                  dram["w_in_xT"][k * 128:(k + 1) * 128, :])
            conv_diag = [p12.tile([128, 128], BF16, name=f"cvd{i}",
                                  tag=f"cvd{i}")
                         for i in range(NDT_FULL * D_CONV)]
            for i in range(NDT_FULL * D_CONV):
                nc.sync.dma_start(conv_diag[i][:],
                                  dram["conv_diag"][i * 128:(i + 1) * 128, :])

            # ---- stages 1+2 fused per d-tile: in-proj -> conv -> silu ----
            for r in range(NDT_FULL):
                xr = p12.tile([128, LPAD], BF16, name="xr", tag="xr",
                              bufs=2)
                nc.vector.memset(xr[:, 0:3], 0.0)
                for n in range(NCH):
                    ps = psum_tile()
                    for k in range(NK):
                        nc.tensor.matmul(
                            ps[:], w_in_xT[k][:, r * 128:(r + 1) * 128],
                            uT[k][:, n * CW:(n + 1) * CW],
                            start=(k == 0), stop=(k == NK - 1))
                    nc.vector.tensor_copy(
                        xr[:, 3 + n * CW:3 + (n + 1) * CW], ps[:])
                xc_dst = xc_own[r] if r < NDT else xc_oth[r - NDT]
                for n in range(NCH):
                    ps = psum_tile()
                    for j in range(D_CONV):
                        nc.tensor.matmul(
                            ps[:], conv_diag[r * D_CONV + j][:],
                            xr[:, n * CW + j:n * CW + j + CW],
                            start=(j == 0), stop=(j == D_CONV - 1))
                    nc.scalar.activation(xc_dst[:, n * CW:(n + 1) * CW],
                                         ps[:], AF.Silu,
                                         bias=conv_b[r][:], scale=1.0)

        # ---- stage 3: x_dbl over the full d_inner (no collective) ----
        for n in range(NCH):
            ps = psum_tile(NXP)
            for k in range(NDT_FULL):
                src = xc_own[k] if k < NDT else xc_oth[k - NDT]
                nc.tensor.matmul(ps[:], w_xT[k][:],
                                 src[:, n * CW:(n + 1) * CW],
                                 start=(k == 0), stop=(k == NDT_FULL - 1))
            nc.scalar.copy(xdbl_bf[:, n * CW:(n + 1) * CW], ps[:])

        nc.sync.dma_start(bc_scr[:], xdbl_bf[64:NXP, :])
        if dbg == 1:
            xdbg = p3.tile([NXP, L], F32, name="xdbg", tag="xdbg")
            nc.vector.tensor_copy(xdbg[:], xdbl_bf[:])
            nc.sync.dma_start(dram["xdbl_dbg"][:], xdbg[:])

        # ---- z half -> silu(z) straight from PSUM ----
        for r in range(NDT):
            for n in range(NCH):
                ps = psum_tile()
                for k in range(NK):
                    nc.tensor.matmul(
                        ps[:], w_in_zT[k][:, r * 128:(r + 1) * 128],
                        uT[k][:, n * CW:(n + 1) * CW],
                        start=(k == 0), stop=(k == NK - 1))
                nc.scalar.activation(gz[r][:, n * CW:(n + 1) * CW],
                                     ps[:], AF.Silu)

        if dbg == 1:
            for r in range(NDT_FULL):
                src = xc_own[r] if r < NDT else xc_oth[r - NDT]
                xcd = p3.tile([128, L], F32, name="xcd", tag="xcd", bufs=2)
                nc.vector.tensor_copy(xcd[:], src[:])
                nc.sync.dma_start(dram["xc_dbg"][r * 128:(r + 1) * 128, :],
                                  xcd[:])


def _scan_stage(nc, tc, dram, dbg, wp, env):
    xdbl_bf = env["xdbl_bf"]
    yg_bf = env["yg_bf"]
    xc_own = env["xc_own"]
    gz = env["gz"]
    bc_scr = env["bc_scr"]
    w_dtT = env["w_dtT"]
    A_col = env["A_col"]
    b_dt = env["b_dt"]
    dp_diag = env["dp_diag"]
    idn = env["idn"]
    psall = env["psall"]
    dtT_bf = xdbl_bf[0:DT_RANK, :]

    with tc.tile_pool(name="scanp", bufs=1) as sp:
        # ---- pre-phase: mdelta[r] = -softplus(dt @ W_dt.T + b_dt)
        # = ln(sigmoid(-x)) for all 6 r (host negates b_dt, A, and the
        # B rows of W_x to absorb the sign; whole path on ScalarE) ----
        mdelta = [sp.tile([128, L], BF16, name=f"md{r}", tag=f"md{r}")
                  for r in range(NDT)]
        for r in range(NDT):
            for n in range(NCH):
                ps = psall.tile([128, CW], F32, name="psd", tag=f"yp{n}")
                nc.tensor.matmul(ps[:], w_dtT[:, r * 128:(r + 1) * 128],
                                 dtT_bf[:, n * CW:(n + 1) * CW],
                                 start=True, stop=True)
                sig = sp.tile([128, CW], F32, name="sig", tag="sig",
                              bufs=2)
                nc.scalar.activation(sig[:], ps[:], AF.Sigmoid,
                                     bias=b_dt[r][:], scale=-1.0)
                nc.scalar.activation(mdelta[r][:, n * CW:(n + 1) * CW],
                                     sig[:], AF.Ln)
            if dbg == 1:
                dd = sp.tile([128, L], F32, name="dd", tag="dd")
                nc.vector.tensor_copy(dd[:], mdelta[r][:])
                nc.sync.dma_start(
                    dram["delta_dbg"][r * 128:(r + 1) * 128, :], dd[:])

        # ---- r-pairs share each state's B/C broadcast tiles ----
        for p in range(NDT // 2):
            r0, r1 = 2 * p, 2 * p + 1
            du = {}
            for r in (r0, r1):
                du[r] = sp.tile([128, L], BF16, name="du", tag="du",
                                bufs=2)
                nc.vector.tensor_tensor(du[r][:], mdelta[r][:],
                                        xc_own[r][:], OP.mult)
            yp = {r0: [psall.tile([128, CW], F32, name="yp", tag=f"yp{n}")
                       for n in range(NCH)],
                  r1: [psall.tile([128, CW], F32, name="yp",
                                  tag=f"yp{NCH + n}")
                       for n in range(NCH)]}

            for s in range(D_STATE):
                b_rep = sp.tile([128, L], BF16, name="b_rep", tag="b_rep",
                                bufs=3)
                nc.sync.dma_start(
                    b_rep[:], bc_scr[s:s + 1, :].broadcast_to((128, L)))
                c_rep = sp.tile([128, L], BF16, name="c_rep", tag="c_rep",
                                bufs=3)
                nc.sync.dma_start(
                    c_rep[:], bc_scr[D_STATE + s:D_STATE + s + 1, :]
                    .broadcast_to((128, L)))
                for r in (r0, r1):
                    dA = sp.tile([128, L], BF16, name="eb", tag="eb",
                                 bufs=3)
                    nc.scalar.activation(dA[:], mdelta[r][:], AF.Exp,
                                         bias=0.0,
                                         scale=A_col[r][:, s:s + 1])
                    dbu = sp.tile([128, L], BF16, name="dbu", tag="dbu",
                                  bufs=3)
                    nc.vector.tensor_tensor(dbu[:], du[r][:], b_rep[:],
                                            OP.mult)
                    h = sp.tile([128, L], BF16, name="h", tag="h", bufs=3)
                    nc.vector.tensor_tensor_scan(h[:], dA[:], dbu[:], 0.0,
                                                 OP.mult, OP.add)
                    ws = sp.tile([128, L], BF16, name="ws", tag="ws",
                                 bufs=2)
                    nc.vector.tensor_tensor(ws[:], h[:], c_rep[:], OP.mult)
                    for n in range(NCH):
                        nc.tensor.matmul(yp[r][n][:], idn[:],
                                         ws[:, n * CW:(n + 1) * CW],
                                         start=(s == 0), stop=False)
            for r in (r0, r1):
                # skip term
                for n in range(NCH):
                    nc.tensor.matmul(yp[r][n][:], dp_diag[r][:],
                                     xc_own[r][:, n * CW:(n + 1) * CW],
                                     start=False, stop=True)
                # gate with silu(z)
                for n in range(NCH):
                    nc.vector.tensor_tensor(
                        yg_bf[r][:, n * CW:(n + 1) * CW], yp[r][n][:],
                        gz[r][:, n * CW:(n + 1) * CW], OP.mult)


# ======================= host side =======================

def _prep_core_inputs(inputs, b, rev, h):
    hs = np.asarray(inputs["hidden_states"])
    W_in = np.asarray(inputs["W_in"])
    conv_w = np.asarray(inputs["conv_w"])[:, 0, :]
    conv_b = np.asarray(inputs["conv_b"])
    W_x = np.asarray(inputs["W_x"])
    W_dt = np.asarray(inputs["W_dt"])
    b_dt = np.asarray(inputs["b_dt"])
    A = -np.exp(np.asarray(inputs["A_log"], np.float64)).astype(np.float32)
    Dp = np.asarray(inputs["Dp"])
    W_out = np.asarray(inputs["W_out"])

    lo, hi = h * HALF, (h + 1) * HALF
    perm = np.r_[lo:hi, (0 if h else HALF):(HALF if h else D_INNER)]

    u = hs[b]
    if rev:
        u = u[::-1]
    uT = np.ascontiguousarray(u.T).astype(BF_NP)

    W_in_x = W_in[0:D_INNER][perm]
    W_in_z = W_in[D_INNER + lo:D_INNER + hi]
    conv_wp = conv_w[perm]
    conv_bp = conv_b[perm].reshape(-1, 1).astype(np.float32)
    W_xp = W_x[:, perm]
    W_xpad = np.zeros((NXP, W_xp.shape[1]), W_xp.dtype)
    W_xpad[0:DT_RANK] = W_xp[0:DT_RANK]
    # B rows negated: device uses mdelta = -delta, so du = -delta*xc and
    # (-B)*du = delta*xc*B. C rows (80:96) keep their sign.
    W_xpad[64:80] = -W_xp[DT_RANK:DT_RANK + D_STATE]
    W_xpad[80:96] = W_xp[DT_RANK + D_STATE:NXD]

    conv_diag = np.zeros((NDT_FULL * D_CONV * 128, 128), np.float32)
    idx = np.arange(128)
    for r in range(NDT_FULL):
        for j in range(D_CONV):
            base = (r * D_CONV + j) * 128
            conv_diag[base + idx, idx] = conv_wp[r * 128:(r + 1) * 128, j]

    dp_diag = np.zeros((NDT * 128, 128), np.float32)
    for r in range(NDT):
        dp_diag[r * 128 + idx, idx] = Dp[lo + r * 128: lo + (r + 1) * 128]

    return {
        "uT": uT,
        "w_in_xT": np.ascontiguousarray(W_in_x.T).astype(BF_NP),
        "w_in_zT": np.ascontiguousarray(W_in_z.T).astype(BF_NP),
        "conv_diag": conv_diag.astype(BF_NP),
        "conv_b": conv_bp,
        "w_xT": np.ascontiguousarray(W_xpad.T).astype(BF_NP),
        "w_dtT": np.ascontiguousarray(W_dt[lo:hi].T).astype(BF_NP),
        "b_dt": -b_dt[lo:hi].reshape(-1, 1).astype(np.float32),
        "A_half": -A[lo:hi].astype(np.float32),
        "dp_diag": dp_diag.astype(BF_NP),
        "idn": np.eye(128, dtype=np.float32).astype(BF_NP),
        "w_outT": np.ascontiguousarray(W_out[:, lo:hi].T).astype(BF_NP),
    }


_CACHE = {}


def kernel(**inputs):
    if "prog" not in _CACHE:
        _CACHE["prog"] = build_program(0)
    nc = _CACHE["prog"]

    in_maps = []
    for c in range(8):
        b, rev, h = c >> 2, (c >> 1) & 1, c & 1
        in_maps.append(_prep_core_inputs(inputs, b, rev, h))
    res = run_bass_kernel_spmd(nc, in_maps, list(range(8)))

    out = np.zeros((BATCH, L, D_MODEL), np.float32)
    for c in range(8):
        b, rev, h = c >> 2, (c >> 1) & 1, c & 1
        part = res.results[c]["out_part"].T
        if rev:
            part = part[::-1]
        out[b] += part
    return out


# revision 21
# speedup vs baseline: 1.2866x; 1.2866x over previous
"""BiMamba forward kernel for 8 TRN2 NeuronCores.

Sharding: core c = (batch b, direction dir, d_inner half h); the host
pre-flips reverse-direction inputs in time so the device program is
identical (purely causal) on all cores. Each core produces a partial
output projection [d_model, L]; the host sums four partials per batch
element (unflipping the reverse ones). A host-side channel permutation
puts this core's d_inner half in x-path tiles 0..5 so the single SPMD
program needs no per-core branches.

Device layout: channels on partitions, time on the free dim. Each core
computes the in-proj/conv for the FULL d_inner locally (PE has slack)
so x_dbl needs no collective. The scan is hardware tensor_tensor_scan
(h = dA*h + dBu along time), one instruction per (128-channel tile,
state s). All elementwise work stays on VectorE: running tensor_tensor
on GpSimd concurrently halves DVE throughput (shared SBUF ports), so
the Pool engine is left idle on purpose. Decay planes dA_s come from
ScalarE exp (per-partition scale = A[:,s]); delta is a single Softplus
activation straight from PSUM. sum_s + Dp-skip accumulate in PSUM via
identity/diagonal matmuls on TensorE; the causal conv also runs on
TensorE as 4 diagonal matmuls over shifted views. B/C rows bounce
through DRAM and return as partition-broadcast DMA reads forming
[128, L] replicated tiles.
"""
import numpy as np
import ml_dtypes

import concourse.bass as bass
import concourse.tile as tile
from concourse import bacc, mybir
from concourse.bass_utils import run_bass_kernel_spmd

D_MODEL = 768
D_INNER = 1536
D_STATE = 16
D_CONV = 4
DT_RANK = 48
BATCH = 2
SEQLEN = 2048

HALF = D_INNER // 2
NDT = HALF // 128            # 6 half d-tiles
NDT_FULL = D_INNER // 128    # 12 full d-tiles
NK = D_MODEL // 128          # 6 k-tiles over d_model
L = SEQLEN
NCH = 4
CW = L // NCH                # 512
NXD = DT_RANK + 2 * D_STATE  # 80
NXP = 96                     # x_dbl psum rows padded: B/C at partition 64
NM = D_MODEL // 128          # 6 out-proj row tiles

F32 = mybir.dt.float32
BF16 = mybir.dt.bfloat16
BF_NP = ml_dtypes.bfloat16

# packed bf16 weight layout (columns)
CV_OFF = 0                                   # conv_diag: 48 x 128
IDN_OFF = CV_OFF + NDT_FULL * D_CONV * 128   # idn: 128
DPD_OFF = IDN_OFF + 128                      # dp_diag: 6 x 128
WOUT_OFF = DPD_OFF + NDT * 128               # w_outT: 6 x 768
WX_OFF = WOUT_OFF + NDT * D_MODEL            # w_xT: 12 x 96
WPACK_BF = WX_OFF + NDT_FULL * NXP
# packed f32 layout (columns)
ACOL_OFF = 0                                 # A_col: 6 x 16
BDT_OFF = ACOL_OFF + NDT * D_STATE           # b_dt: 6 x 1
CVB_OFF = BDT_OFF + NDT                      # conv_b: 12 x 1
WPACK_F32 = CVB_OFF + NDT_FULL

AF = mybir.ActivationFunctionType
OP = mybir.AluOpType


def build_program(debug_stage=0):
    nc = bacc.Bacc("TRN2", target_bir_lowering=False, debug=False,
                   num_devices=8)
    dram = {}

    def din(name, shape, dt):
        dram[name] = nc.dram_tensor(name, list(shape), dt,
                                    kind="ExternalInput").ap()

    def dout(name, shape, dt):
        dram[name] = nc.dram_tensor(name, list(shape), dt,
                                    kind="ExternalOutput").ap()

    din("uT", (D_MODEL, L), BF16)
    din("w_in_xT", (D_MODEL, D_INNER), BF16)
    din("w_in_zT", (D_MODEL, HALF), BF16)
    din("w_dtT", (DT_RANK, HALF), BF16)
    # packed small weights: conv_diag | idn | dp_diag | w_outT | w_xT
    din("wpack_bf", (128, WPACK_BF), BF16)
    # packed f32 columns: A_col | b_dt | conv_b
    din("wpack_f32", (128, WPACK_F32), F32)

    if debug_stage == 1:
        dout("xc_dbg", (D_INNER, L), F32)
        dout("delta_dbg", (HALF, L), F32)
        dout("xdbl_dbg", (NXP, L), F32)
    for pi in range(NDT // 2):
        dout(f"out_part{pi}", (D_MODEL, L), F32)

    with tile.TileContext(nc) as tc:
        _body_once(nc, tc, dram, debug_stage)
    nc.compile()
    return nc


def _body_once(nc, tc, dram, dbg):
    with tc.tile_pool(name="wpool", bufs=1) as wp, \
         tc.tile_pool(name="dramp", bufs=1, space="DRAM") as dp_pool:

        # ---- DRAM scratch for B/C partition-broadcast bounce ----
        bc_scr = dp_pool.tile([2 * D_STATE, L], BF16, name="bc_scr")

        # ---- persistent small weights: two packed loads + views ----
        wpack = wp.tile([128, WPACK_BF], BF16, name="wpack")
        nc.sync.dma_start(wpack[:], dram["wpack_bf"][:])
        fpack = wp.tile([128, WPACK_F32], F32, name="fpack")
        nc.sync.dma_start(fpack[:], dram["wpack_f32"][:])
        idn = wpack[:, IDN_OFF:IDN_OFF + 128]
        dp_diag = [wpack[:, DPD_OFF + r * 128:DPD_OFF + (r + 1) * 128]
                   for r in range(NDT)]
        w_outT = [wpack[:, WOUT_OFF + r * D_MODEL:
                        WOUT_OFF + (r + 1) * D_MODEL]
                  for r in range(NDT)]
        w_xT = [wpack[:, WX_OFF + k * NXP:WX_OFF + (k + 1) * NXP]
                for k in range(NDT_FULL)]
        conv_diag = [wpack[:, CV_OFF + i * 128:CV_OFF + (i + 1) * 128]
                     for i in range(NDT_FULL * D_CONV)]
        A_col = [fpack[:, ACOL_OFF + r * D_STATE:
                       ACOL_OFF + (r + 1) * D_STATE]
                 for r in range(NDT)]
        b_dt = [fpack[:, BDT_OFF + r:BDT_OFF + r + 1] for r in range(NDT)]
        conv_b = [fpack[:, CVB_OFF + r:CVB_OFF + r + 1]
                  for r in range(NDT_FULL)]
        w_dtT = wp.tile([DT_RANK, HALF], BF16, name="w_dtT")
        nc.sync.dma_start(w_dtT[:], dram["w_dtT"][:])

        with tc.tile_pool(name="hold", bufs=1) as hold:
            xdbl_bf = hold.tile([NXP, L], BF16, name="xdbl_bf")
            yg_bf = [hold.tile([128, L], BF16, name=f"yg{r}")
                     for r in range(NDT)]
            xc_own = [hold.tile([128, L], BF16, name=f"xco{r}")
                      for r in range(NDT)]
            gz = [hold.tile([128, L], BF16, name=f"gz{r}")
                  for r in range(NDT)]

            with tc.tile_pool(name="psall", bufs=1,
                              space="PSUM") as psall:
                env = {"hold": hold, "xdbl_bf": xdbl_bf, "yg_bf": yg_bf,
                       "xc_own": xc_own, "gz": gz, "bc_scr": bc_scr,
                       "conv_b": conv_b, "w_xT": w_xT, "w_dtT": w_dtT,
                       "A_col": A_col, "b_dt": b_dt, "dp_diag": dp_diag,
                       "idn": idn, "psall": psall, "w_outT": w_outT,
                       "conv_diag": conv_diag}
                _stages_123(nc, tc, dram, dbg, wp, env)
                _scan_stage(nc, tc, dram, dbg, wp, env)


def _stages_123(nc, tc, dram, dbg, wp, env):
    xdbl_bf = env["xdbl_bf"]
    xc_own = env["xc_own"]
    gz = env["gz"]
    conv_b = env["conv_b"]
    w_xT = env["w_xT"]
    bc_scr = env["bc_scr"]
    psall = env["psall"]
    LPAD = L + 3
    psn = [0]

    def psum_tile(rows=128):
        t = psall.tile([rows, CW], F32, name="psp",
                       tag=f"yp{psn[0] % 4}")
        psn[0] += 1
        return t

    with tc.tile_pool(name="pre3", bufs=1) as p3:
        xc_oth = [p3.tile([128, L], BF16, name=f"xoth{r}", tag=f"xoth{r}")
                  for r in range(NDT_FULL - NDT)]
        uT = [p3.tile([128, L], BF16, name=f"uT{k}", tag=f"uT{k}")
              for k in range(NK)]
        for k in range(NK):
            nc.sync.dma_start(uT[k][:],
                              dram["uT"][k * 128:(k + 1) * 128, :])
        w_in_zT = [p3.tile([128, HALF], BF16, name=f"wiz{k}",
                           tag=f"wiz{k}") for k in range(NK)]
        for k in range(NK):
            nc.sync.dma_start(w_in_zT[k][:],
                              dram["w_in_zT"][k * 128:(k + 1) * 128, :])
        with tc.tile_pool(name="pre12", bufs=1) as p12:
            w_in_xT = [p12.tile([128, D_INNER], BF16, name=f"wix{k}",
                                tag=f"wix{k}") for k in range(NK)]
            for k in range(NK):
                nc.sync.dma_start(w_in_xT[k][:],
                                  dram["w_in_xT"][k * 128:(k + 1) * 128, :])
            conv_diag = env["conv_diag"]

            # ---- stages 1+2 fused per d-tile: in-proj -> conv -> silu ----
            for r in range(NDT_FULL):
                xr = p12.tile([128, LPAD], BF16, name="xr", tag="xr",
                              bufs=2)
                nc.vector.memset(xr[:, 0:3], 0.0)
                for n in range(NCH):
                    ps = psum_tile()
                    for k in range(NK):
                        nc.tensor.matmul(
                            ps[:], w_in_xT[k][:, r * 128:(r + 1) * 128],
                            uT[k][:, n * CW:(n + 1) * CW],
                            start=(k == 0), stop=(k == NK - 1))
                    nc.vector.tensor_copy(
                        xr[:, 3 + n * CW:3 + (n + 1) * CW], ps[:])
                xc_dst = xc_own[r] if r < NDT else xc_oth[r - NDT]
                for n in range(NCH):
                    ps = psum_tile()
                    for j in range(D_CONV):
                        nc.tensor.matmul(
                            ps[:], conv_diag[r * D_CONV + j][:],
                            xr[:, n * CW + j:n * CW + j + CW],
                            start=(j == 0), stop=(j == D_CONV - 1))
                    nc.scalar.activation(xc_dst[:, n * CW:(n + 1) * CW],
                                         ps[:], AF.Silu,
                                         bias=conv_b[r][:], scale=1.0)

        # ---- stage 3: x_dbl over the full d_inner (no collective) ----
        for n in range(NCH):
            ps = psum_tile(NXP)
            for k in range(NDT_FULL):
                src = xc_own[k] if k < NDT else xc_oth[k - NDT]
                nc.tensor.matmul(ps[:], w_xT[k][:],
                                 src[:, n * CW:(n + 1) * CW],
                                 start=(k == 0), stop=(k == NDT_FULL - 1))
            nc.scalar.copy(xdbl_bf[:, n * CW:(n + 1) * CW], ps[:])

        nc.sync.dma_start(bc_scr[:], xdbl_bf[64:NXP, :])
        if dbg == 1:
            xdbg = p3.tile([NXP, L], F32, name="xdbg", tag="xdbg")
            nc.vector.tensor_copy(xdbg[:], xdbl_bf[:])
            nc.sync.dma_start(dram["xdbl_dbg"][:], xdbg[:])

        # ---- z half -> silu(z) straight from PSUM ----
        for r in range(NDT):
            for n in range(NCH):
                ps = psum_tile()
                for k in range(NK):
                    nc.tensor.matmul(
                        ps[:], w_in_zT[k][:, r * 128:(r + 1) * 128],
                        uT[k][:, n * CW:(n + 1) * CW],
                        start=(k == 0), stop=(k == NK - 1))
                nc.scalar.activation(gz[r][:, n * CW:(n + 1) * CW],
                                     ps[:], AF.Silu)

        if dbg == 1:
            for r in range(NDT_FULL):
                src = xc_own[r] if r < NDT else xc_oth[r - NDT]
                xcd = p3.tile([128, L], F32, name="xcd", tag="xcd", bufs=2)
                nc.vector.tensor_copy(xcd[:], src[:])
                nc.sync.dma_start(dram["xc_dbg"][r * 128:(r + 1) * 128, :],
                                  xcd[:])


def _scan_stage(nc, tc, dram, dbg, wp, env):
    xdbl_bf = env["xdbl_bf"]
    yg_bf = env["yg_bf"]
    xc_own = env["xc_own"]
    gz = env["gz"]
    bc_scr = env["bc_scr"]
    w_dtT = env["w_dtT"]
    A_col = env["A_col"]
    b_dt = env["b_dt"]
    dp_diag = env["dp_diag"]
    idn = env["idn"]
    psall = env["psall"]
    dtT_bf = xdbl_bf[0:DT_RANK, :]

    w_outT = env["w_outT"]

    with tc.tile_pool(name="scanp", bufs=1) as sp:
        for r in range(NDT):
            # ---- mdelta = -softplus(dt @ W_dt.T + b_dt) = ln(sigmoid(-x))
            # (host negates b_dt, A, and the B rows of W_x to absorb the
            # sign; the whole delta path stays on ScalarE) ----
            sig = sp.tile([128, L], F32, name="sig", tag="sig")
            for n in range(NCH):
                ps = psall.tile([128, CW], F32, name="psd", tag="dt",
                                bufs=2)
                nc.tensor.matmul(ps[:], w_dtT[:, r * 128:(r + 1) * 128],
                                 dtT_bf[:, n * CW:(n + 1) * CW],
                                 start=True, stop=True)
                nc.scalar.activation(sig[:, n * CW:(n + 1) * CW], ps[:],
                                     AF.Sigmoid, bias=b_dt[r][:],
                                     scale=-1.0)
            mdelta = sp.tile([128, L], BF16, name="mdelta", tag="mdelta",
                             bufs=2)
            nc.scalar.activation(mdelta[:], sig[:], AF.Ln)
            if dbg == 1:
                dd = sp.tile([128, L], F32, name="dd", tag="dd")
                nc.vector.tensor_copy(dd[:], mdelta[:])
                nc.sync.dma_start(
                    dram["delta_dbg"][r * 128:(r + 1) * 128, :], dd[:])

            # ---- du = mdelta * xc (sign fixed by negated B rows) ----
            du = sp.tile([128, L], BF16, name="du", tag="du", bufs=2)
            nc.vector.tensor_tensor(du[:], mdelta[:], xc_own[r][:], OP.mult)

            yp = [psall.tile([128, CW], F32, name="yp", tag=f"yp{n}")
                  for n in range(NCH)]

            for s in range(D_STATE):
                dA = sp.tile([128, L], BF16, name="eb", tag="eb", bufs=3)
                nc.scalar.activation(dA[:], mdelta[:], AF.Exp, bias=0.0,
                                     scale=A_col[r][:, s:s + 1])
                b_rep = sp.tile([128, L], BF16, name="b_rep", tag="b_rep",
                                bufs=3)
                nc.sync.dma_start(
                    b_rep[:], bc_scr[s:s + 1, :].broadcast_to((128, L)))
                c_rep = sp.tile([128, L], BF16, name="c_rep", tag="c_rep",
                                bufs=3)
                nc.sync.dma_start(
                    c_rep[:], bc_scr[D_STATE + s:D_STATE + s + 1, :]
                    .broadcast_to((128, L)))
                dbu = sp.tile([128, L], BF16, name="dbu", tag="dbu",
                              bufs=3)
                nc.vector.tensor_tensor(dbu[:], du[:], b_rep[:], OP.mult)
                h = sp.tile([128, L], BF16, name="h", tag="h", bufs=3)
                nc.vector.tensor_tensor_scan(h[:], dA[:], dbu[:], 0.0,
                                             OP.mult, OP.add)
                ws = sp.tile([128, L], BF16, name="ws", tag="ws", bufs=3)
                nc.vector.tensor_tensor(ws[:], h[:], c_rep[:], OP.mult)
                for n in range(NCH):
                    nc.tensor.matmul(yp[n][:], idn[:],
                                     ws[:, n * CW:(n + 1) * CW],
                                     start=(s == 0), stop=False)
            # skip term
            for n in range(NCH):
                nc.tensor.matmul(yp[n][:], dp_diag[r][:],
                                 xc_own[r][:, n * CW:(n + 1) * CW],
                                 start=False, stop=True)
            # gate with silu(z)
            for n in range(NCH):
                nc.vector.tensor_tensor(yg_bf[r][:, n * CW:(n + 1) * CW],
                                        yp[n][:],
                                        gz[r][:, n * CW:(n + 1) * CW],
                                        OP.mult)

            # ---- partial out-proj per r-pair: overlaps later r's scans;
            # the host sums the three partials ----
            if r % 2 == 1:
                pi = r // 2
                for m in range(NM):
                    for n in range(NCH):
                        ps = psall.tile([128, CW], F32, name="pso",
                                        tag="op", bufs=2)
                        for rr in (r - 1, r):
                            nc.tensor.matmul(
                                ps[:], w_outT[rr][:, m * 128:(m + 1) * 128],
                                yg_bf[rr][:, n * CW:(n + 1) * CW],
                                start=(rr == r - 1), stop=(rr == r))
                        ot = sp.tile([128, CW], F32, name="ot", tag="ot",
                                     bufs=4)
                        nc.scalar.copy(ot[:], ps[:])
                        nc.sync.dma_start(
                            dram[f"out_part{pi}"][m * 128:(m + 1) * 128,
                                                  n * CW:(n + 1) * CW],
                            ot[:])


# ======================= host side =======================

def _prep_core_inputs(inputs, b, rev, h):
    hs = np.asarray(inputs["hidden_states"])
    W_in = np.asarray(inputs["W_in"])
    conv_w = np.asarray(inputs["conv_w"])[:, 0, :]
    conv_b = np.asarray(inputs["conv_b"])
    W_x = np.asarray(inputs["W_x"])
    W_dt = np.asarray(inputs["W_dt"])
    b_dt = np.asarray(inputs["b_dt"])
    A = -np.exp(np.asarray(inputs["A_log"], np.float64)).astype(np.float32)
    Dp = np.asarray(inputs["Dp"])
    W_out = np.asarray(inputs["W_out"])

    lo, hi = h * HALF, (h + 1) * HALF
    perm = np.r_[lo:hi, (0 if h else HALF):(HALF if h else D_INNER)]

    u = hs[b]
    if rev:
        u = u[::-1]
    uT = np.ascontiguousarray(u.T).astype(BF_NP)

    W_in_x = W_in[0:D_INNER][perm]
    W_in_z = W_in[D_INNER + lo:D_INNER + hi]
    conv_wp = conv_w[perm]
    conv_bp = conv_b[perm].reshape(-1, 1).astype(np.float32)
    W_xp = W_x[:, perm]
    W_xpad = np.zeros((NXP, W_xp.shape[1]), W_xp.dtype)
    W_xpad[0:DT_RANK] = W_xp[0:DT_RANK]
    # B rows negated: device uses mdelta = -delta, so du = -delta*xc and
    # (-B)*du = delta*xc*B. C rows (80:96) keep their sign.
    W_xpad[64:80] = -W_xp[DT_RANK:DT_RANK + D_STATE]
    W_xpad[80:96] = W_xp[DT_RANK + D_STATE:NXD]

    idx = np.arange(128)
    wpack = np.zeros((128, WPACK_BF), np.float32)
    for r in range(NDT_FULL):
        for j in range(D_CONV):
            base = CV_OFF + (r * D_CONV + j) * 128
            wpack[idx, base + idx] = conv_wp[r * 128:(r + 1) * 128, j]
    wpack[idx, IDN_OFF + idx] = 1.0
    for r in range(NDT):
        wpack[idx, DPD_OFF + r * 128 + idx] = \
            Dp[lo + r * 128: lo + (r + 1) * 128]
    w_outT = W_out[:, lo:hi].T  # (HALF, D_MODEL)
    for r in range(NDT):
        wpack[:, WOUT_OFF + r * D_MODEL:WOUT_OFF + (r + 1) * D_MODEL] = \
            w_outT[r * 128:(r + 1) * 128]
    w_xT = W_xpad.T  # (HALF*2, NXP)
    for k in range(NDT_FULL):
        wpack[:, WX_OFF + k * NXP:WX_OFF + (k + 1) * NXP] = \
            w_xT[k * 128:(k + 1) * 128]

    fpack = np.zeros((128, WPACK_F32), np.float32)
    for r in range(NDT):
        fpack[:, ACOL_OFF + r * D_STATE:ACOL_OFF + (r + 1) * D_STATE] = \
            -A[lo + r * 128:lo + (r + 1) * 128]
        fpack[:, BDT_OFF + r] = -b_dt[lo + r * 128:lo + (r + 1) * 128]
    for r in range(NDT_FULL):
        fpack[:, CVB_OFF + r] = conv_bp[r * 128:(r + 1) * 128, 0]

    return {
        "uT": uT,
        "w_in_xT": np.ascontiguousarray(W_in_x.T).astype(BF_NP),
        "w_in_zT": np.ascontiguousarray(W_in_z.T).astype(BF_NP),
        "w_dtT": np.ascontiguousarray(W_dt[lo:hi].T).astype(BF_NP),
        "wpack_bf": wpack.astype(BF_NP),
        "wpack_f32": fpack,
    }


_CACHE = {}


def kernel(**inputs):
    if "prog" not in _CACHE:
        _CACHE["prog"] = build_program(0)
    nc = _CACHE["prog"]

    in_maps = []
    for c in range(8):
        b, rev, h = c >> 2, (c >> 1) & 1, c & 1
        in_maps.append(_prep_core_inputs(inputs, b, rev, h))
    res = run_bass_kernel_spmd(nc, in_maps, list(range(8)))

    out = np.zeros((BATCH, L, D_MODEL), np.float32)
    for c in range(8):
        b, rev, h = c >> 2, (c >> 1) & 1, c & 1
        part = (res.results[c]["out_part0"]
                + res.results[c]["out_part1"]
                + res.results[c]["out_part2"]).T
        if rev:
            part = part[::-1]
        out[b] += part
    return out
